# revision 12
# baseline (speedup 1.0000x reference)
"""Diversity7 loss kernel for Trainium2 (8 NeuronCores, Bass/Tile).

Math (per batch row b):
  p_m   = softmax(x_m / T)                          m = 0..6, C = 1000 classes
  v_m   = (p_m - mean(p_m)) / ||p_m - mean(p_m)||   (mean(p_m) = 1/C exactly)
  q_b   = || sum_m v_m ||^2
  loss  = SCALE * mean_b((q_b - M) / 2)

Device-side restructuring (all f32):
  e    = exp(x/T)                  (ACT pass, accum_out gives Se = sum e)
  dev2 = sum_c (e - Se/C)^2        (centered -> no catastrophic cancellation;
                                    split between ACT Square+accum and DVE
                                    affine_mul_reduce to balance engines)
  r2   = dev2/Se^2;  inv_r = exp(-0.5*ln(r2)) + one Newton step
  g    = inv_r/Se;   h = -inv_r/C
  v_m  = g*e + h  (in place over e, on GpSimd)   == centered normalized v_m
  s    = sum_m v_m                  (tensor adds, DVE/GpSimd)
  q    = sum_c s^2                  (fused affine_mul_reduce, DVE)
Host finishes in f64: loss = SCALE * mean((q-7)/2).

Sharding: data-parallel over batch. 8 cores x 512 rows; each core sees
[512,1000] slices of the 7 logit tensors and emits q for its rows as [128,4]
(partition p, row-tile rt) -> global row = core*512 + rt*128 + p.
`targets` is accepted and ignored (unused by the reference loss).
"""

import sys

import numpy as np

if "/opt/trn_rl_repo" not in sys.path:
    sys.path.insert(0, "/opt/trn_rl_repo")

import concourse.bass as bass
import concourse.tile as tile
from concourse import bacc, mybir
from concourse.bass_utils import run_bass_kernel_spmd

T = 20.0
SCALE = 0.3
C = 1000
M = 7
N_CORES = 8
ROWS_PER_CORE = 512
RT = ROWS_PER_CORE // 128  # row-tiles per core
MT = M * RT  # 28 (model, row-tile) pairs

# Engine balance tunables
ACT_DEV2_MODELS = (0, 1)  # m values whose dev2 runs on ACT (per rt)
POOL_TT_PER_RT = 2        # how many of the 6 s-chain adds go to GpSimd

F32 = mybir.dt.float32
AF = mybir.ActivationFunctionType
ALU = mybir.AluOpType


def _is_act_dev2(m: int) -> bool:
    return m in ACT_DEV2_MODELS


def _build_program() -> bass.Bass:
    nc = bacc.Bacc()
    xs = [
        nc.declare_dram_parameter(f"x{m}", [ROWS_PER_CORE, C], F32, isOutput=False)
        for m in range(M)
    ]
    # Per-column rescale for dev2: ACT columns hold +sum(e-eb)^2, DVE columns
    # hold -C*sum(e-eb)*e; colscale is 1.0 / -1/C respectively.
    colscale_in = nc.declare_dram_parameter("colscale", [128, MT], F32, isOutput=False)
    q_out = nc.declare_dram_parameter("q_out", [128, RT], F32, isOutput=True)

    with tile.TileContext(nc) as tc:
        with (
            tc.tile_pool(name="xp", bufs=4) as xp,
            tc.tile_pool(name="ep", bufs=1) as ep,
            tc.tile_pool(name="sp", bufs=2) as sp,
            tc.tile_pool(name="trp", bufs=2) as trp,
            tc.tile_pool(name="smp", bufs=1) as smp,
            tc.tile_pool(name="qp", bufs=1) as qp,
        ):
            q = qp.tile([128, RT], F32)
            colscale = smp.tile([128, MT], F32, tag="colscale")
            nc.sync.dma_start(colscale[:], colscale_in[:])
            Se = smp.tile([128, MT], F32, tag="Se")
            dev2 = smp.tile([128, MT], F32, tag="dev2")

            # Phase 1: exp + row-sum + centered square-sum for all 28 tiles.
            es: dict[int, bass.AP] = {}
            for rt in range(RT):
                for m in range(M):
                    k = rt * M + m
                    x = xp.tile([128, C], F32, tag="x")
                    nc.sync.dma_start(x[:], xs[m][rt * 128 : (rt + 1) * 128, :])
                    e = ep.tile([128, C], F32, tag=f"e{k}")
                    nc.scalar.activation(
                        e[:], x[:], AF.Exp, bias=0.0, scale=1.0 / T,
                        accum_out=Se[:, k : k + 1],
                    )
                    trash = trp.tile([128, C], F32, tag="trash")
                    if _is_act_dev2(m):
                        # dev2 = sum (e - Se/C)^2 on ACT. The rounded -1/C in
                        # negSeC only enters quadratically here (sum(e-eb)=0).
                        negSeC = smp.tile([128, 1], F32, tag=f"negSeC{k}")
                        nc.gpsimd.tensor_scalar_mul(
                            negSeC[:], Se[:, k : k + 1], -1.0 / C
                        )
                        nc.scalar.activation(
                            trash[:], e[:], AF.Square, bias=negSeC[:], scale=1.0,
                            accum_out=dev2[:, k : k + 1],
                        )
                    else:
                        # accum = sum (-C*e + Se)*e == -C*(Se2 - Se^2/C);
                        # scale/bias exact in f32, 1/C applied via colscale.
                        nc.vector.affine_mul_reduce(
                            out=trash[:], accum_out=dev2[:, k : k + 1],
                            in0=e[:], in1=e[:], scale=-float(C),
                            bias=Se[:, k : k + 1],
                        )
                    es[k] = e

            # Phase 2: per-row scalars, batched over all 28 columns.
            invSe = smp.tile([128, MT], F32, tag="invSe")
            nc.vector.reciprocal(invSe[:], Se[:])
            t0 = smp.tile([128, MT], F32, tag="t0")
            nc.vector.tensor_tensor(t0[:], dev2[:], invSe[:], ALU.mult)
            t1 = smp.tile([128, MT], F32, tag="t1")
            nc.vector.tensor_tensor(t1[:], t0[:], invSe[:], ALU.mult)
            r2 = smp.tile([128, MT], F32, tag="r2")
            nc.vector.tensor_tensor(r2[:], t1[:], colscale[:], ALU.mult)
            # rsqrt seed via ln/exp (same ACT table set), then one Newton step
            lnr = smp.tile([128, MT], F32, tag="lnr")
            nc.scalar.activation(lnr[:], r2[:], AF.Ln)
            invr0 = smp.tile([128, MT], F32, tag="invr0")
            nc.scalar.activation(invr0[:], lnr[:], AF.Exp, bias=0.0, scale=-0.5)
            y0sq = smp.tile([128, MT], F32, tag="y0sq")
            nc.vector.tensor_tensor(y0sq[:], invr0[:], invr0[:], ALU.mult)
            zy = smp.tile([128, MT], F32, tag="zy")
            nc.vector.tensor_tensor(zy[:], r2[:], y0sq[:], ALU.mult)
            nrc = smp.tile([128, MT], F32, tag="nrc")
            nc.vector.tensor_scalar(
                nrc[:], zy[:], -0.5, 1.5, op0=ALU.mult, op1=ALU.add
            )
            invr = smp.tile([128, MT], F32, tag="invr")
            nc.vector.tensor_tensor(invr[:], invr0[:], nrc[:], ALU.mult)
            g = smp.tile([128, MT], F32, tag="g")
            nc.vector.tensor_tensor(g[:], invr[:], invSe[:], ALU.mult)
            h = smp.tile([128, MT], F32, tag="h")
            nc.vector.tensor_scalar_mul(h[:], invr[:], -1.0 / C)

            # Phase 3: v_k = g*e + h in place over e (GpSimd), then
            # s = sum_m v_m with adds split DVE/GpSimd, then q = sum s^2.
            for rt in range(RT):
                for m in range(M):
                    k = rt * M + m
                    nc.gpsimd.tensor_scalar(
                        es[k][:], es[k][:], g[:, k : k + 1], h[:, k : k + 1],
                        op0=ALU.mult, op1=ALU.add,
                    )
                s_prev = None
                for m in range(1, M):
                    k = rt * M + m
                    s_new = sp.tile([128, C], F32, tag="s")
                    in0 = es[rt * M][:] if m == 1 else s_prev[:]
                    eng = nc.gpsimd if m <= POOL_TT_PER_RT else nc.vector
                    eng.tensor_tensor(s_new[:], in0, es[k][:], ALU.add)
                    s_prev = s_new
                trash2 = trp.tile([128, C], F32, tag="trash")
                nc.vector.affine_mul_reduce(
                    out=trash2[:], accum_out=q[:, rt : rt + 1],
                    in0=s_prev[:], in1=s_prev[:], scale=1.0, bias=0.0,
                )
            nc.sync.dma_start(q_out[:], q[:])
    return nc


_NC_CACHE: bass.Bass | None = None


def _get_program() -> bass.Bass:
    global _NC_CACHE
    if _NC_CACHE is None:
        nc = _build_program()
        nc.finalize()
        _NC_CACHE = nc
    return _NC_CACHE


def _colscale_np() -> np.ndarray:
    row = np.empty((MT,), dtype=np.float32)
    for rt in range(RT):
        for m in range(M):
            row[rt * M + m] = 1.0 if _is_act_dev2(m) else -1.0 / C
    return np.broadcast_to(row, (128, MT)).copy()


def run_device_part(inputs: dict[str, np.ndarray], **run_kwargs):
    """Run the bass kernel; returns (q_all [4096] f64 row-major, results)."""
    nc = _get_program()
    core_ids = list(range(N_CORES))
    colscale = _colscale_np()
    in_maps = []
    for c in range(N_CORES):
        lo, hi = c * ROWS_PER_CORE, (c + 1) * ROWS_PER_CORE
        im = {
            f"x{m}": np.ascontiguousarray(
                inputs[f"outputs{m + 1}"][lo:hi], dtype=np.float32
            )
            for m in range(M)
        }
        im["colscale"] = colscale
        in_maps.append(im)
    res = run_bass_kernel_spmd(nc, in_maps, core_ids, **run_kwargs)
    qs = []
    for c in range(N_CORES):
        qc = np.asarray(res.results[c]["q_out"])  # [128, RT]
        qs.append(qc.T.reshape(-1))  # row = rt*128 + p order
    q_all = np.concatenate(qs).astype(np.float64)  # row = c*512 + rt*128 + p
    return q_all, res


def kernel(**inputs: np.ndarray) -> np.ndarray:
    q_all, _ = run_device_part(inputs)
    loss = SCALE * np.mean((q_all - float(M)) / 2.0)
    return np.float32(loss)


# revision 14
# speedup vs baseline: 1.0059x; 1.0059x over previous
"""Diversity7 loss kernel for Trainium2 (8 NeuronCores, Bass/Tile).

Math (per batch row b):
  p_m   = softmax(x_m / T)                          m = 0..6, C = 1000 classes
  v_m   = (p_m - mean(p_m)) / ||p_m - mean(p_m)||   (mean(p_m) = 1/C exactly)
  q_b   = || sum_m v_m ||^2
  loss  = SCALE * mean_b((q_b - M) / 2)

Device-side restructuring (all f32):
  e    = exp(x/T)                  (ACT pass, accum_out gives Se = sum e)
  dev2 = sum_c (e - Se/C)^2        (centered -> no catastrophic cancellation;
                                    split between ACT Square+accum and DVE
                                    affine_mul_reduce to balance engines)
  r2   = dev2/Se^2;  inv_r = exp(-0.5*ln(r2)) + one Newton step
  g    = inv_r/Se;   h = -inv_r/C
  v_m  = g*e + h  (in place over e, on GpSimd)   == centered normalized v_m
  s    = sum_m v_m                  (tensor adds, DVE/GpSimd)
  q    = sum_c s^2                  (fused affine_mul_reduce, DVE)
Host finishes in f64: loss = SCALE * mean((q-7)/2).

Sharding: data-parallel over batch. 8 cores x 512 rows; each core sees
[512,1000] slices of the 7 logit tensors and emits q for its rows as [128,4]
(partition p, row-tile rt) -> global row = core*512 + rt*128 + p.
`targets` is accepted and ignored (unused by the reference loss).
"""

import sys

import numpy as np

if "/opt/trn_rl_repo" not in sys.path:
    sys.path.insert(0, "/opt/trn_rl_repo")

import concourse.bass as bass
import concourse.tile as tile
from concourse import bacc, mybir
from concourse.bass_utils import run_bass_kernel_spmd

T = 20.0
SCALE = 0.3
C = 1000
M = 7
N_CORES = 8
ROWS_PER_CORE = 512
RT = ROWS_PER_CORE // 128  # row-tiles per core
MT = M * RT  # 28 (model, row-tile) pairs

# Engine balance tunables
ACT_DEV2_MODELS = (0, 1)  # m values whose dev2 runs on ACT (per rt)
POOL_TT_PER_RT = 5        # how many of the 6 s-chain adds go to GpSimd

F32 = mybir.dt.float32
AF = mybir.ActivationFunctionType
ALU = mybir.AluOpType


def _is_act_dev2(m: int) -> bool:
    return m in ACT_DEV2_MODELS


def _build_program() -> bass.Bass:
    nc = bacc.Bacc()
    xs = [
        nc.declare_dram_parameter(f"x{m}", [ROWS_PER_CORE, C], F32, isOutput=False)
        for m in range(M)
    ]
    # Per-column rescale for dev2: ACT columns hold +sum(e-eb)^2, DVE columns
    # hold -C*sum(e-eb)*e; colscale is 1.0 / -1/C respectively.
    colscale_in = nc.declare_dram_parameter("colscale", [128, MT], F32, isOutput=False)
    q_out = nc.declare_dram_parameter("q_out", [128, RT], F32, isOutput=True)

    with tile.TileContext(nc) as tc:
        with (
            tc.tile_pool(name="xp", bufs=4) as xp,
            tc.tile_pool(name="ep", bufs=1) as ep,
            tc.tile_pool(name="sp", bufs=2) as sp,
            tc.tile_pool(name="trp", bufs=2) as trp,
            tc.tile_pool(name="smp", bufs=1) as smp,
            tc.tile_pool(name="qp", bufs=1) as qp,
        ):
            q = qp.tile([128, RT], F32)
            colscale = smp.tile([128, MT], F32, tag="colscale")
            nc.sync.dma_start(colscale[:], colscale_in[:])
            Se = smp.tile([128, MT], F32, tag="Se")
            dev2 = smp.tile([128, MT], F32, tag="dev2")

            # Fully interleaved per row-tile so phases of different row-tiles
            # overlap across engines.
            for rt in range(RT):
                sl = slice(rt * M, (rt + 1) * M)
                es: list[bass.AP] = []
                for m in range(M):
                    k = rt * M + m
                    x = xp.tile([128, C], F32, tag="x")
                    nc.sync.dma_start(x[:], xs[m][rt * 128 : (rt + 1) * 128, :])
                    e = ep.tile([128, C], F32, tag=f"e{m}", bufs=2)
                    nc.scalar.activation(
                        e[:], x[:], AF.Exp, bias=0.0, scale=1.0 / T,
                        accum_out=Se[:, k : k + 1],
                    )
                    trash = trp.tile([128, C], F32, tag="trash")
                    if _is_act_dev2(m):
                        # dev2 = sum (e - Se/C)^2 on ACT. The rounded -1/C in
                        # negSeC only enters quadratically (sum(e-eb) == 0).
                        negSeC = smp.tile([128, 1], F32, tag=f"negSeC{k}")
                        nc.gpsimd.tensor_scalar_mul(
                            negSeC[:], Se[:, k : k + 1], -1.0 / C
                        )
                        nc.scalar.activation(
                            trash[:], e[:], AF.Square, bias=negSeC[:], scale=1.0,
                            accum_out=dev2[:, k : k + 1],
                        )
                    else:
                        # accum = sum (-C*e + Se)*e == -C*(Se2 - Se^2/C);
                        # scale/bias exact in f32, 1/C applied via colscale.
                        nc.vector.affine_mul_reduce(
                            out=trash[:], accum_out=dev2[:, k : k + 1],
                            in0=e[:], in1=e[:], scale=-float(C),
                            bias=Se[:, k : k + 1],
                        )
                    es.append(e)

                # Per-row scalars for this row-tile ([128, 7] slices).
                invSe = smp.tile([128, M], F32, tag="invSe")
                nc.vector.reciprocal(invSe[:], Se[:, sl])
                t0 = smp.tile([128, M], F32, tag="t0")
                nc.vector.tensor_tensor(t0[:], dev2[:, sl], invSe[:], ALU.mult)
                t1 = smp.tile([128, M], F32, tag="t1")
                nc.vector.tensor_tensor(t1[:], t0[:], invSe[:], ALU.mult)
                r2 = smp.tile([128, M], F32, tag="r2")
                nc.vector.tensor_tensor(r2[:], t1[:], colscale[:, sl], ALU.mult)
                # rsqrt seed via ln/exp (both live in the natural_log_exp ACT
                # table set together with Exp/Square), then one Newton step.
                lnr = smp.tile([128, M], F32, tag="lnr")
                nc.scalar.activation(lnr[:], r2[:], AF.Ln)
                invr0 = smp.tile([128, M], F32, tag="invr0")
                nc.scalar.activation(invr0[:], lnr[:], AF.Exp, bias=0.0, scale=-0.5)
                y0sq = smp.tile([128, M], F32, tag="y0sq")
                nc.vector.tensor_tensor(y0sq[:], invr0[:], invr0[:], ALU.mult)
                zy = smp.tile([128, M], F32, tag="zy")
                nc.vector.tensor_tensor(zy[:], r2[:], y0sq[:], ALU.mult)
                nrc = smp.tile([128, M], F32, tag="nrc")
                nc.vector.tensor_scalar(
                    nrc[:], zy[:], -0.5, 1.5, op0=ALU.mult, op1=ALU.add
                )
                invr = smp.tile([128, M], F32, tag="invr")
                nc.vector.tensor_tensor(invr[:], invr0[:], nrc[:], ALU.mult)
                g = smp.tile([128, M], F32, tag="g")
                nc.vector.tensor_tensor(g[:], invr[:], invSe[:], ALU.mult)
                h = smp.tile([128, M], F32, tag="h")
                nc.vector.tensor_scalar_mul(h[:], invr[:], -1.0 / C)

                # v_m = g*e + h in place over e (DVE ts runs at 2x), then
                # s = sum_m v_m with adds split DVE/GpSimd, q = sum s^2.
                for m in range(M):
                    nc.vector.tensor_scalar(
                        es[m][:], es[m][:], g[:, m : m + 1], h[:, m : m + 1],
                        op0=ALU.mult, op1=ALU.add,
                    )
                s_prev = None
                for m in range(1, M):
                    s_new = sp.tile([128, C], F32, tag="s")
                    in0 = es[0][:] if m == 1 else s_prev[:]
                    eng = nc.gpsimd if m <= POOL_TT_PER_RT else nc.vector
                    eng.tensor_tensor(s_new[:], in0, es[m][:], ALU.add)
                    s_prev = s_new
                trash2 = trp.tile([128, C], F32, tag="trash")
                nc.vector.affine_mul_reduce(
                    out=trash2[:], accum_out=q[:, rt : rt + 1],
                    in0=s_prev[:], in1=s_prev[:], scale=1.0, bias=0.0,
                )
            nc.sync.dma_start(q_out[:], q[:])
    return nc


_NC_CACHE: bass.Bass | None = None


def _get_program() -> bass.Bass:
    global _NC_CACHE
    if _NC_CACHE is None:
        nc = _build_program()
        nc.finalize()
        _NC_CACHE = nc
    return _NC_CACHE


def _colscale_np() -> np.ndarray:
    row = np.empty((MT,), dtype=np.float32)
    for rt in range(RT):
        for m in range(M):
            row[rt * M + m] = 1.0 if _is_act_dev2(m) else -1.0 / C
    return np.broadcast_to(row, (128, MT)).copy()


def run_device_part(inputs: dict[str, np.ndarray], **run_kwargs):
    """Run the bass kernel; returns (q_all [4096] f64 row-major, results)."""
    nc = _get_program()
    core_ids = list(range(N_CORES))
    colscale = _colscale_np()
    in_maps = []
    for c in range(N_CORES):
        lo, hi = c * ROWS_PER_CORE, (c + 1) * ROWS_PER_CORE
        im = {
            f"x{m}": np.ascontiguousarray(
                inputs[f"outputs{m + 1}"][lo:hi], dtype=np.float32
            )
            for m in range(M)
        }
        im["colscale"] = colscale
        in_maps.append(im)
    res = run_bass_kernel_spmd(nc, in_maps, core_ids, **run_kwargs)
    qs = []
    for c in range(N_CORES):
        qc = np.asarray(res.results[c]["q_out"])  # [128, RT]
        qs.append(qc.T.reshape(-1))  # row = rt*128 + p order
    q_all = np.concatenate(qs).astype(np.float64)  # row = c*512 + rt*128 + p
    return q_all, res


def kernel(**inputs: np.ndarray) -> np.ndarray:
    q_all, _ = run_device_part(inputs)
    loss = SCALE * np.mean((q_all - float(M)) / 2.0)
    return np.float32(loss)


# revision 17
# speedup vs baseline: 1.0881x; 1.0817x over previous
"""Diversity7 loss kernel for Trainium2 (8 NeuronCores, Bass/Tile).

Math (per batch row b):
  p_m   = softmax(x_m / T)                          m = 0..6, C = 1000 classes
  v_m   = (p_m - mean(p_m)) / ||p_m - mean(p_m)||   (mean(p_m) = 1/C exactly)
  q_b   = || sum_m v_m ||^2
  loss  = SCALE * mean_b((q_b - M) / 2)

Device-side restructuring (all f32):
  e    = exp(x/T)                  (ACT pass, accum_out gives Se = sum e)
  dev2 = sum_c (e - Se/C)^2        (centered -> no catastrophic cancellation;
                                    split between ACT Square+accum and DVE
                                    affine_mul_reduce to balance engines)
  r2   = dev2/Se^2;  inv_r = exp(-0.5*ln(r2)) + one Newton step
  g    = inv_r/Se;   h = -inv_r/C
  v_m  = g*e + h  (in place over e, on GpSimd)   == centered normalized v_m
  s    = sum_m v_m                  (tensor adds, DVE/GpSimd)
  q    = sum_c s^2                  (fused affine_mul_reduce, DVE)
Host finishes in f64: loss = SCALE * mean((q-7)/2).

Sharding: data-parallel over batch. 8 cores x 512 rows; each core sees
[512,1000] slices of the 7 logit tensors and emits q for its rows as [128,4]
(partition p, row-tile rt) -> global row = core*512 + rt*128 + p.
`targets` is accepted and ignored (unused by the reference loss).
"""

import sys

import numpy as np

if "/opt/trn_rl_repo" not in sys.path:
    sys.path.insert(0, "/opt/trn_rl_repo")

import concourse.bass as bass
import concourse.tile as tile
from concourse import bacc, mybir
from concourse.bass_utils import run_bass_kernel_spmd


def _patch_act_tables() -> None:
    """Make Exp/Ln/Square resolve only via natural_log_exp_and_others so the
    kernel needs exactly one ACT table load (the default first-fit choice
    thrashes exp_and_others <-> natural_log sets, ~1.3us per switch)."""
    import concourse.hw_specs as hw_specs

    if getattr(hw_specs, "_diversity7_patched", False):
        return
    orig = hw_specs.get_activation_tables

    def patched(module_arch):
        tables = orig(module_arch)
        keep = "natural_log_exp_and_others"
        if keep in tables:
            only = {
                mybir.ActivationFunctionType.Exp,
                mybir.ActivationFunctionType.Ln,
                mybir.ActivationFunctionType.Square,
            }
            for name, funcs in tables.items():
                if name != keep:
                    funcs -= only
        return tables

    hw_specs.get_activation_tables = patched
    bacc.get_activation_tables = patched
    hw_specs._diversity7_patched = True

T = 20.0
SCALE = 0.3
C = 1000
M = 7
N_CORES = 8
ROWS_PER_CORE = 512
RT = ROWS_PER_CORE // 128  # row-tiles per core
MT = M * RT  # 28 (model, row-tile) pairs

# Engine balance tunables
ACT_DEV2_MODELS = (0, 1)  # m values whose dev2 runs on ACT (per rt)
POOL_TT_PER_RT = 5        # how many of the 6 s-chain adds go to GpSimd

F32 = mybir.dt.float32
AF = mybir.ActivationFunctionType
ALU = mybir.AluOpType


def _is_act_dev2(m: int) -> bool:
    return m in ACT_DEV2_MODELS


def _build_program() -> bass.Bass:
    _patch_act_tables()
    nc = bacc.Bacc()
    xs = [
        nc.declare_dram_parameter(f"x{m}", [ROWS_PER_CORE, C], F32, isOutput=False)
        for m in range(M)
    ]
    # Per-column rescale for dev2: ACT columns hold +sum(e-eb)^2, DVE columns
    # hold -C*sum(e-eb)*e; colscale is 1.0 / -1/C respectively.
    colscale_in = nc.declare_dram_parameter("colscale", [128, MT], F32, isOutput=False)
    q_out = nc.declare_dram_parameter("q_out", [128, RT], F32, isOutput=True)

    with tile.TileContext(nc) as tc:
        with (
            tc.tile_pool(name="xp", bufs=4) as xp,
            tc.tile_pool(name="ep", bufs=1) as ep,
            tc.tile_pool(name="sp", bufs=2) as sp,
            tc.tile_pool(name="trp", bufs=2) as trp,
            tc.tile_pool(name="smp", bufs=1) as smp,
            tc.tile_pool(name="qp", bufs=1) as qp,
        ):
            q = qp.tile([128, RT], F32)
            colscale = smp.tile([128, MT], F32, tag="colscale")
            nc.sync.dma_start(colscale[:], colscale_in[:])
            Se = smp.tile([128, MT], F32, tag="Se")
            dev2 = smp.tile([128, MT], F32, tag="dev2")

            # Fully interleaved per row-tile so phases of different row-tiles
            # overlap across engines.
            for rt in range(RT):
                sl = slice(rt * M, (rt + 1) * M)
                es: list[bass.AP] = []
                for m in range(M):
                    k = rt * M + m
                    x = xp.tile([128, C], F32, tag="x")
                    nc.sync.dma_start(x[:], xs[m][rt * 128 : (rt + 1) * 128, :])
                    e = ep.tile([128, C], F32, tag=f"e{m}", bufs=2)
                    nc.scalar.activation(
                        e[:], x[:], AF.Exp, bias=0.0, scale=1.0 / T,
                        accum_out=Se[:, k : k + 1],
                    )
                    trash = trp.tile([128, C], F32, tag="trash")
                    if _is_act_dev2(m):
                        # dev2 = sum (e - Se/C)^2 on ACT. The rounded -1/C in
                        # negSeC only enters quadratically (sum(e-eb) == 0).
                        negSeC = smp.tile([128, 1], F32, tag=f"negSeC{k}")
                        nc.gpsimd.tensor_scalar_mul(
                            negSeC[:], Se[:, k : k + 1], -1.0 / C
                        )
                        nc.scalar.activation(
                            trash[:], e[:], AF.Square, bias=negSeC[:], scale=1.0,
                            accum_out=dev2[:, k : k + 1],
                        )
                    else:
                        # accum = sum (-C*e + Se)*e == -C*(Se2 - Se^2/C);
                        # scale/bias exact in f32, 1/C applied via colscale.
                        nc.vector.affine_mul_reduce(
                            out=trash[:], accum_out=dev2[:, k : k + 1],
                            in0=e[:], in1=e[:], scale=-float(C),
                            bias=Se[:, k : k + 1],
                        )
                    es.append(e)

                # Per-row scalars for this row-tile ([128, 7] slices).
                invSe = smp.tile([128, M], F32, tag="invSe")
                nc.vector.reciprocal(invSe[:], Se[:, sl])
                t0 = smp.tile([128, M], F32, tag="t0")
                nc.vector.tensor_tensor(t0[:], dev2[:, sl], invSe[:], ALU.mult)
                t1 = smp.tile([128, M], F32, tag="t1")
                nc.vector.tensor_tensor(t1[:], t0[:], invSe[:], ALU.mult)
                r2 = smp.tile([128, M], F32, tag="r2")
                nc.vector.tensor_tensor(r2[:], t1[:], colscale[:, sl], ALU.mult)
                # rsqrt seed via ln/exp (both live in the natural_log_exp ACT
                # table set together with Exp/Square), then one Newton step.
                lnr = smp.tile([128, M], F32, tag="lnr")
                nc.scalar.activation(lnr[:], r2[:], AF.Ln)
                invr0 = smp.tile([128, M], F32, tag="invr0")
                nc.scalar.activation(invr0[:], lnr[:], AF.Exp, bias=0.0, scale=-0.5)
                y0sq = smp.tile([128, M], F32, tag="y0sq")
                nc.vector.tensor_tensor(y0sq[:], invr0[:], invr0[:], ALU.mult)
                zy = smp.tile([128, M], F32, tag="zy")
                nc.vector.tensor_tensor(zy[:], r2[:], y0sq[:], ALU.mult)
                nrc = smp.tile([128, M], F32, tag="nrc")
                nc.vector.tensor_scalar(
                    nrc[:], zy[:], -0.5, 1.5, op0=ALU.mult, op1=ALU.add
                )
                invr = smp.tile([128, M], F32, tag="invr")
                nc.vector.tensor_tensor(invr[:], invr0[:], nrc[:], ALU.mult)
                g = smp.tile([128, M], F32, tag="g")
                nc.vector.tensor_tensor(g[:], invr[:], invSe[:], ALU.mult)
                h = smp.tile([128, M], F32, tag="h")
                nc.vector.tensor_scalar_mul(h[:], invr[:], -1.0 / C)

                # v_m = g*e + h (DVE ts at 2x needs out != in), then
                # s = sum_m v_m with adds split DVE/GpSimd, q = sum s^2.
                vs: list[bass.AP] = []
                for m in range(M):
                    v = sp.tile([128, C], F32, tag=f"v{m}", bufs=2)
                    nc.vector.tensor_scalar(
                        v[:], es[m][:], g[:, m : m + 1], h[:, m : m + 1],
                        op0=ALU.mult, op1=ALU.add,
                    )
                    vs.append(v)
                s_prev = None
                for m in range(1, M):
                    s_new = sp.tile([128, C], F32, tag="s")
                    in0 = vs[0][:] if m == 1 else s_prev[:]
                    eng = nc.gpsimd if m <= POOL_TT_PER_RT else nc.vector
                    eng.tensor_tensor(s_new[:], in0, vs[m][:], ALU.add)
                    s_prev = s_new
                trash2 = trp.tile([128, C], F32, tag="trash")
                nc.vector.affine_mul_reduce(
                    out=trash2[:], accum_out=q[:, rt : rt + 1],
                    in0=s_prev[:], in1=s_prev[:], scale=1.0, bias=0.0,
                )
            nc.sync.dma_start(q_out[:], q[:])
    return nc


_NC_CACHE: bass.Bass | None = None


def _get_program() -> bass.Bass:
    global _NC_CACHE
    if _NC_CACHE is None:
        nc = _build_program()
        nc.finalize()
        _NC_CACHE = nc
    return _NC_CACHE


def _colscale_np() -> np.ndarray:
    row = np.empty((MT,), dtype=np.float32)
    for rt in range(RT):
        for m in range(M):
            row[rt * M + m] = 1.0 if _is_act_dev2(m) else -1.0 / C
    return np.broadcast_to(row, (128, MT)).copy()


def run_device_part(inputs: dict[str, np.ndarray], **run_kwargs):
    """Run the bass kernel; returns (q_all [4096] f64 row-major, results)."""
    nc = _get_program()
    core_ids = list(range(N_CORES))
    colscale = _colscale_np()
    in_maps = []
    for c in range(N_CORES):
        lo, hi = c * ROWS_PER_CORE, (c + 1) * ROWS_PER_CORE
        im = {
            f"x{m}": np.ascontiguousarray(
                inputs[f"outputs{m + 1}"][lo:hi], dtype=np.float32
            )
            for m in range(M)
        }
        im["colscale"] = colscale
        in_maps.append(im)
    res = run_bass_kernel_spmd(nc, in_maps, core_ids, **run_kwargs)
    qs = []
    for c in range(N_CORES):
        qc = np.asarray(res.results[c]["q_out"])  # [128, RT]
        qs.append(qc.T.reshape(-1))  # row = rt*128 + p order
    q_all = np.concatenate(qs).astype(np.float64)  # row = c*512 + rt*128 + p
    return q_all, res


def kernel(**inputs: np.ndarray) -> np.ndarray:
    q_all, _ = run_device_part(inputs)
    loss = SCALE * np.mean((q_all - float(M)) / 2.0)
    return np.float32(loss)


# revision 19
# speedup vs baseline: 1.3577x; 1.2477x over previous
"""Diversity7 loss kernel for Trainium2 (8 NeuronCores, Bass/Tile).

Math (per batch row b):
  p_m   = softmax(x_m / T)                          m = 0..6, C = 1000 classes
  v_m   = (p_m - mean(p_m)) / ||p_m - mean(p_m)||   (mean(p_m) = 1/C exactly)
  q_b   = || sum_m v_m ||^2
  loss  = SCALE * mean_b((q_b - M) / 2)

Device-side restructuring (all f32):
  e    = exp(x/T)                  (ACT pass, accum_out gives Se = sum e)
  dev2 = sum_c (e - Se/C)^2        (centered -> no catastrophic cancellation;
                                    split between ACT Square+accum and DVE
                                    affine_mul_reduce to balance engines)
  r2   = dev2/Se^2;  inv_r = exp(-0.5*ln(r2)) + one Newton step
  g    = inv_r/Se;   h = -inv_r/C
  v_m  = g*e + h  (in place over e, on GpSimd)   == centered normalized v_m
  s    = sum_m v_m                  (tensor adds, DVE/GpSimd)
  q    = sum_c s^2                  (fused affine_mul_reduce, DVE)
Host finishes in f64: loss = SCALE * mean((q-7)/2).

Sharding: data-parallel over batch. 8 cores x 512 rows; each core sees
[512,1000] slices of the 7 logit tensors and emits q for its rows as [128,4]
(partition p, row-tile rt) -> global row = core*512 + rt*128 + p.
`targets` is accepted and ignored (unused by the reference loss).
"""

import sys

import numpy as np

if "/opt/trn_rl_repo" not in sys.path:
    sys.path.insert(0, "/opt/trn_rl_repo")

import concourse.bass as bass
import concourse.tile as tile
from concourse import bacc, mybir
from concourse.bass_utils import run_bass_kernel_spmd


def _patch_act_tables() -> None:
    """Make Exp/Ln/Square resolve only via natural_log_exp_and_others so the
    kernel needs exactly one ACT table load (the default first-fit choice
    thrashes exp_and_others <-> natural_log sets, ~1.3us per switch)."""
    import concourse.hw_specs as hw_specs

    if getattr(hw_specs, "_diversity7_patched", False):
        return
    orig = hw_specs.get_activation_tables

    def patched(module_arch):
        tables = orig(module_arch)
        keep = "natural_log_exp_and_others"
        if keep in tables:
            only = {
                mybir.ActivationFunctionType.Exp,
                mybir.ActivationFunctionType.Ln,
                mybir.ActivationFunctionType.Square,
            }
            for name, funcs in tables.items():
                if name != keep:
                    funcs -= only
        return tables

    hw_specs.get_activation_tables = patched
    bacc.get_activation_tables = patched
    hw_specs._diversity7_patched = True

T = 20.0
SCALE = 0.3
C = 1000
M = 7
N_CORES = 8
ROWS_PER_CORE = 512
RT = ROWS_PER_CORE // 128  # row-tiles per core
MT = M * RT  # 28 (model, row-tile) pairs

# Engine balance tunables. GpSimd is kept OFF the big [128,1000] ops: its
# SBUF traffic contends with DVE's read ports and slows DVE ~2x (measured
# tensor_scalar 800ns -> 1475ns when pool runs big tensor_tensor underneath).
ACT_DEV2_MODELS = (0, 1, 2)  # m values whose dev2 runs on ACT (per rt)

F32 = mybir.dt.float32
AF = mybir.ActivationFunctionType
ALU = mybir.AluOpType


def _is_act_dev2(m: int) -> bool:
    return m in ACT_DEV2_MODELS


def _build_program() -> bass.Bass:
    _patch_act_tables()
    nc = bacc.Bacc()
    xs = [
        nc.declare_dram_parameter(f"x{m}", [ROWS_PER_CORE, C], F32, isOutput=False)
        for m in range(M)
    ]
    # Per-column rescale for dev2: ACT columns hold +sum(e-eb)^2, DVE columns
    # hold -C*sum(e-eb)*e; colscale is 1.0 / -1/C respectively.
    colscale_in = nc.declare_dram_parameter("colscale", [128, MT], F32, isOutput=False)
    q_out = nc.declare_dram_parameter("q_out", [128, RT], F32, isOutput=True)

    with tile.TileContext(nc) as tc:
        with (
            tc.tile_pool(name="xp", bufs=4) as xp,
            tc.tile_pool(name="ep", bufs=1) as ep,
            tc.tile_pool(name="sp", bufs=2) as sp,
            tc.tile_pool(name="trp", bufs=2) as trp,
            tc.tile_pool(name="smp", bufs=1) as smp,
            tc.tile_pool(name="qp", bufs=1) as qp,
        ):
            q = qp.tile([128, RT], F32)
            colscale = smp.tile([128, MT], F32, tag="colscale")
            nc.sync.dma_start(colscale[:], colscale_in[:])
            Se = smp.tile([128, MT], F32, tag="Se")
            dev2 = smp.tile([128, MT], F32, tag="dev2")

            # Fully interleaved per row-tile so phases of different row-tiles
            # overlap across engines.
            for rt in range(RT):
                sl = slice(rt * M, (rt + 1) * M)
                es: list[bass.AP] = []
                for m in range(M):
                    k = rt * M + m
                    x = xp.tile([128, C], F32, tag="x")
                    nc.sync.dma_start(x[:], xs[m][rt * 128 : (rt + 1) * 128, :])
                    e = ep.tile([128, C], F32, tag=f"e{m}", bufs=2)
                    nc.scalar.activation(
                        e[:], x[:], AF.Exp, bias=0.0, scale=1.0 / T,
                        accum_out=Se[:, k : k + 1],
                    )
                    trash = trp.tile([128, C], F32, tag="trash")
                    if _is_act_dev2(m):
                        # dev2 = sum (e - Se/C)^2 on ACT. The rounded -1/C in
                        # negSeC only enters quadratically (sum(e-eb) == 0).
                        negSeC = smp.tile([128, 1], F32, tag=f"negSeC{k}")
                        nc.gpsimd.tensor_scalar_mul(
                            negSeC[:], Se[:, k : k + 1], -1.0 / C
                        )
                        nc.scalar.activation(
                            trash[:], e[:], AF.Square, bias=negSeC[:], scale=1.0,
                            accum_out=dev2[:, k : k + 1],
                        )
                    else:
                        # accum = sum (-C*e + Se)*e == -C*(Se2 - Se^2/C);
                        # scale/bias exact in f32, 1/C applied via colscale.
                        nc.vector.affine_mul_reduce(
                            out=trash[:], accum_out=dev2[:, k : k + 1],
                            in0=e[:], in1=e[:], scale=-float(C),
                            bias=Se[:, k : k + 1],
                        )
                    es.append(e)

                # Per-row scalars for this row-tile ([128, 7] slices).
                invSe = smp.tile([128, M], F32, tag="invSe")
                nc.vector.reciprocal(invSe[:], Se[:, sl])
                t0 = smp.tile([128, M], F32, tag="t0")
                nc.vector.tensor_tensor(t0[:], dev2[:, sl], invSe[:], ALU.mult)
                t1 = smp.tile([128, M], F32, tag="t1")
                nc.vector.tensor_tensor(t1[:], t0[:], invSe[:], ALU.mult)
                r2 = smp.tile([128, M], F32, tag="r2")
                nc.vector.tensor_tensor(r2[:], t1[:], colscale[:, sl], ALU.mult)
                # rsqrt seed via ln/exp (both live in the natural_log_exp ACT
                # table set together with Exp/Square), then one Newton step.
                lnr = smp.tile([128, M], F32, tag="lnr")
                nc.scalar.activation(lnr[:], r2[:], AF.Ln)
                invr0 = smp.tile([128, M], F32, tag="invr0")
                nc.scalar.activation(invr0[:], lnr[:], AF.Exp, bias=0.0, scale=-0.5)
                y0sq = smp.tile([128, M], F32, tag="y0sq")
                nc.vector.tensor_tensor(y0sq[:], invr0[:], invr0[:], ALU.mult)
                zy = smp.tile([128, M], F32, tag="zy")
                nc.vector.tensor_tensor(zy[:], r2[:], y0sq[:], ALU.mult)
                nrc = smp.tile([128, M], F32, tag="nrc")
                nc.vector.tensor_scalar(
                    nrc[:], zy[:], -0.5, 1.5, op0=ALU.mult, op1=ALU.add
                )
                invr = smp.tile([128, M], F32, tag="invr")
                nc.vector.tensor_tensor(invr[:], invr0[:], nrc[:], ALU.mult)
                g = smp.tile([128, M], F32, tag="g")
                nc.vector.tensor_tensor(g[:], invr[:], invSe[:], ALU.mult)
                h = smp.tile([128, M], F32, tag="h")
                nc.vector.tensor_scalar_mul(h[:], invr[:], -1.0 / C)

                # s = sum_m (g_m*e_m + h_m) via fused affine_then_add chain;
                # each g*e + h == v_m (centered, normalized).
                s_prev = None
                for m in range(M):
                    s_new = sp.tile([128, C], F32, tag="s")
                    if m == 0:
                        nc.vector.tensor_scalar(
                            s_new[:], es[0][:], g[:, 0:1], h[:, 0:1],
                            op0=ALU.mult, op1=ALU.add,
                        )
                    else:
                        nc.vector.affine_then_add(
                            s_new[:], es[m][:], s_prev[:], g[:, m : m + 1],
                            h[:, m : m + 1],
                        )
                    s_prev = s_new
                trash2 = trp.tile([128, C], F32, tag="trash")
                nc.vector.affine_mul_reduce(
                    out=trash2[:], accum_out=q[:, rt : rt + 1],
                    in0=s_prev[:], in1=s_prev[:], scale=1.0, bias=0.0,
                )
            nc.sync.dma_start(q_out[:], q[:])
    return nc


_NC_CACHE: bass.Bass | None = None


def _get_program() -> bass.Bass:
    global _NC_CACHE
    if _NC_CACHE is None:
        nc = _build_program()
        nc.finalize()
        _NC_CACHE = nc
    return _NC_CACHE


def _colscale_np() -> np.ndarray:
    row = np.empty((MT,), dtype=np.float32)
    for rt in range(RT):
        for m in range(M):
            row[rt * M + m] = 1.0 if _is_act_dev2(m) else -1.0 / C
    return np.broadcast_to(row, (128, MT)).copy()


def run_device_part(inputs: dict[str, np.ndarray], **run_kwargs):
    """Run the bass kernel; returns (q_all [4096] f64 row-major, results)."""
    nc = _get_program()
    core_ids = list(range(N_CORES))
    colscale = _colscale_np()
    in_maps = []
    for c in range(N_CORES):
        lo, hi = c * ROWS_PER_CORE, (c + 1) * ROWS_PER_CORE
        im = {
            f"x{m}": np.ascontiguousarray(
                inputs[f"outputs{m + 1}"][lo:hi], dtype=np.float32
            )
            for m in range(M)
        }
        im["colscale"] = colscale
        in_maps.append(im)
    res = run_bass_kernel_spmd(nc, in_maps, core_ids, **run_kwargs)
    qs = []
    for c in range(N_CORES):
        qc = np.asarray(res.results[c]["q_out"])  # [128, RT]
        qs.append(qc.T.reshape(-1))  # row = rt*128 + p order
    q_all = np.concatenate(qs).astype(np.float64)  # row = c*512 + rt*128 + p
    return q_all, res


def kernel(**inputs: np.ndarray) -> np.ndarray:
    q_all, _ = run_device_part(inputs)
    loss = SCALE * np.mean((q_all - float(M)) / 2.0)
    return np.float32(loss)
